# revision 32
# baseline (speedup 1.0000x reference)
"""Multi-head causal attention (B=2, T=2048, E=1024, H=16, D=64) on 8 TRN2
NeuronCores, tensor-parallel over heads (2 heads per core).

All matmuls run in bf16 (PSUM accumulation f32; ~3e-3 max-rel error vs
the f32 reference). Per 512-t block j:
  A(j) qkv projection -> B(j) scores+AV -> L(j) stash denominator rows ->
  M(j) broadcast l via PE + reciprocal + normalize -> P(j) out-proj + DMA.
Emission: A(j), M(j-1), B(j), P(j-1), L(j); B-order [0,1,2,3,5,6,7,4] so
the smallest attention block absorbs the un-hideable epilogue tail (the
final block uses a per-128-col-quarter pipelined epilogue).

Key performance facts learned from hardware traces (microbenched):
 * 512-col bf16/f32r matmuls sustain ~216 ns back-to-back with LDWEIGHTS
   hidden, BUT K=64 stationaries run at HALF rate (427 ns) and mixing
   (64,128)/(128,128) PE tile sizes is slower still. Scores therefore
   run K=128 against per-head q/k tiles re-based to partition 0 with
   rows 64-127 zeroed once (the pad rows contribute nothing).
 * Causal column-trim: score/AV matmuls and Exp only touch [woff:512];
   below-diagonal chunks are never computed (PSUM cols stay untouched).
 * The B phase is paced by Scalar Exp (~610 ns per (si,h) group vs
   ~430 ns of PE work), so AV matmuls trail their scores by LAG=6 groups
   and the scheduler fills PE idle with neighbor-block A/P matmuls.
 * GPSIMD cannot read PSUM; 'copy' shares the Exp activation table, so a
   slice of the PSUM->SBUF moves (q_h1, 2/8 y copies) runs on Scalar.
 * DVE cost ~ free-size: [1,512] copies cost like [128,512]; reciprocal
   is ~6.5 ns/elem (custom-DVE fast ops don't compile here).

Dataflow per core: xt = x^T [E, BT] bf16 (same on all cores); wqkv_c
[E, 384] (2 heads of Wq|Wk|Wv); wproj_c = Wproj[128c:128c+128, :].
  A: qT,kT,vT [d,t] = wqkv_c^T @ xt (v first; PSUM accum over E tiles);
     v[s,d] via PE transpose, with a ones column per head appended so
     the softmax denominator falls out of the AV matmul (row 64).
  B: per 128-s-tile x head: weiT[s,t] = kT_pad^T q_pad on [woff:512],
     Exp on ACT (scale=E^-0.5) PSUM->SBUF bf16, multiplicative 0/1 tril
     mask on the diagonal chunk (DVE), avT_aug[65,t] += [v|1]^T wt.
  M: bc[128,t] = sel^T @ l rows (PE broadcast, f32r), reciprocal on DVE
     (doubles as the PSUM->SBUF move), two muls -> merged-head lhsT.
  P: y[t,e] = avT^T @ wproj per 128-t chunk; DMA out.
  host: y = sum_c y_c + bproj (tensor-parallel partial sums, f64).

Infra notes: this container's walrus accepts at most ONE semaphore wait
per instruction (_split_multi_waits hoists extras onto EventSemaphores);
the ACT engine must stay on one table set (Exp+Copy); DMA can only be
issued from the Sync/GpSimd/Scalar queues. PSUM rings: qkv(2) + wei(3)
+ av(3) = 8 banks.
"""
import sys
import types

import numpy as np

B, T, E, H, D = 2, 2048, 1024, 16, 64
N_CORES = 8
HPC = H // N_CORES          # heads per core = 2
BT = B * T                  # 4096
DPC = HPC * D               # 128 head-dims per core
SCALE = 1.0 / float(np.sqrt(E))  # NOTE: reference scales by E**-0.5
NB = BT // 512              # 8 global 512-t blocks
NTB = T // 512              # 4 t-blocks per batch
NST = T // 128              # 16 s-tiles per batch
NE = E // 128               # 8 e-tiles


def _install_ntff_hook():
    if 'antenv.axon_hooks' in sys.modules:
        return
    try:
        sys.path.insert(0, '/root/.axon_site')
        from trn_agent_boot.trn_boot import _ntff_profile_via_ctypes
        hook = _ntff_profile_via_ctypes('/opt/axon/libaxon_pjrt.so')
        mod = types.ModuleType('antenv.axon_hooks')
        mod.get_axon_ntff_profile_hook = lambda: hook
        mod.set_axon_ntff_profile_hook = lambda h: None
        sys.modules['antenv.axon_hooks'] = mod
    except Exception:
        pass


def _split_multi_waits(nc, mybir):
    """This walrus build rejects >1 sync-wait per instruction. Hoist extra
    waits onto EventSemaphore instructions on the same engine just before."""
    for f in nc.m.functions:
        for bb in f.blocks:
            new_insts = []
            changed = False
            for inst in bb.instructions:
                si = inst.sync_info
                if si is not None and len(si.on_wait) > 1:
                    extra = list(si.on_wait[:-1])
                    keep = si.on_wait[-1]
                    for w in extra:
                        ev = mybir.InstEventSemaphore(
                            name=f"I-{nc.next_id()}", ins=[], outs=[])
                        ev.engine = inst.engine
                        ev.sync_info = mybir.SyncInfo(on_wait=[w], on_update=[])
                        new_insts.append(ev)
                    del si.on_wait[:]
                    si.on_wait.append(keep)
                    changed = True
                new_insts.append(inst)
            if changed:
                bb.instructions = new_insts


def _build_nc():
    import concourse.bass as bass
    import concourse.mybir as mybir
    import concourse.tile as tile
    from concourse.masks import make_identity

    f32 = mybir.dt.float32
    f32r = mybir.dt.float32r
    bf16 = mybir.dt.bfloat16
    EXP = mybir.ActivationFunctionType.Exp

    nc = bass.Bass('TRN2', num_devices=N_CORES)
    xt = nc.dram_tensor('xt', [E, BT], bf16, kind='ExternalInput')
    wqkv = nc.dram_tensor('wqkv', [E, 3 * DPC], bf16, kind='ExternalInput')
    wproj = nc.dram_tensor('wproj', [DPC, E], bf16, kind='ExternalInput')
    y = nc.dram_tensor('y', [BT, E], f32, kind='ExternalOutput')

    with tile.TileContext(nc) as tc:
        with tc.tile_pool(name='consts', bufs=1) as consts, \
             tc.tile_pool(name='big', bufs=1) as big, \
             tc.tile_pool(name='work', bufs=1) as work, \
             tc.tile_pool(name='ps', bufs=1, space='PSUM') as ps:

            # ---- constants ----
            ident_f = consts.tile([128, 128], f32)
            make_identity(nc, ident_f)
            ident = consts.tile([128, 128], bf16)
            nc.vector.tensor_copy(ident[:], ident_f[:])
            # multiplicative mask for the diagonal chunk of weiT [s,t]:
            # keep (1) where t >= s, 0 where t < s
            tmask_f = consts.tile([128, 128], f32)
            nc.gpsimd.memset(tmask_f[:], 1.0)
            nc.gpsimd.affine_select(
                out=tmask_f[:], in_=tmask_f[:],
                compare_op=mybir.AluOpType.is_ge,
                fill=0.0, base=0, pattern=[[1, 128]], channel_multiplier=-1)
            tmask = consts.tile([128, 128], bf16)
            nc.vector.tensor_copy(tmask[:], tmask_f[:])
            # sel [33,128]: row0 -> partitions 0-63 (head0), row32 -> 64-127
            sel_f32 = consts.tile([33, 128], f32)
            nc.gpsimd.memset(sel_f32[:], 0.0)
            nc.gpsimd.memset(sel_f32[0:1, 0:64], 1.0)
            nc.gpsimd.memset(sel_f32[32:33, 64:128], 1.0)
            sel_bc = consts.tile([33, 128], f32r)   # lhsT of bcast matmul
            nc.vector.tensor_copy(sel_bc[:], sel_f32[:])
            # manually double-buffered denominator rows (only rows 0/32 are
            # ever rewritten; the rest must stay finite for the bcast matmul)
            lrow_t = [consts.tile([33, 512], f32r, name=f'lrow{i}')
                      for i in range(2)]
            for i in range(2):
                nc.gpsimd.memset(lrow_t[i][:].bitcast(mybir.dt.uint32),
                                 1065353216)

            # ---- weights ----
            wqkv_sb = [consts.tile([128, 3 * DPC], bf16, name=f'wqkv{k}')
                       for k in range(NE)]
            dma_qs = [nc.sync, nc.gpsimd, nc.scalar]
            for k in range(NE):
                dma_qs[k % 3].dma_start(out=wqkv_sb[k][:],
                                        in_=wqkv[k * 128:(k + 1) * 128, :])
            wproj_sb = consts.tile([DPC, E], bf16)
            nc.gpsimd.dma_start(out=wproj_sb[:], in_=wproj[:])

            # ---- persistent activations ----
            # per-head q/k tiles re-based to partition 0 with rows 64-127
            # zeroed once: score matmuls run K=128 (full-rate PE tile mode,
            # the zero pad rows contribute nothing)
            qT_sb = [[big.tile([128, 512], bf16, name=f'q{j}_{h}')
                      for h in range(HPC)] for j in range(NB)]
            kT_sb = [[big.tile([128, 512], bf16, name=f'k{j}_{h}')
                      for h in range(HPC)] for j in range(NB)]

            # v tiles [s,d] per 128-s-tile, layout [128, 2, 65]: per head 64
            # dims + ones column (softmax denominator via the AV matmul)
            v_sb = [big.tile([128, 2, 65], bf16, name=f'v{si}')
                    for si in range(2 * NST)]

            # per-block state carried between emission stages
            st = [dict() for _ in range(NB)]

            # ---- A(j): QKV projection for one 512-t block ----
            def emit_A(j):
                ts = j * 512
                # one-time zero/ones fills for this block's pad tiles,
                # spread across the run to keep the startup short
                mseng = nc.vector if j == 0 else nc.gpsimd
                for h in range(HPC):
                    mseng.memset(
                        qT_sb[j][h][64:128, :].bitcast(mybir.dt.uint16), 0)
                    mseng.memset(
                        kT_sb[j][h][64:128, :].bitcast(mybir.dt.uint16), 0)
                for sc in range(4):
                    mseng.memset(
                        v_sb[j * 4 + sc][:, :, 64:65].bitcast(
                            mybir.dt.uint16), 16256)
                xt_t = []
                for k in range(NE):
                    xk = work.tile([128, 512], bf16, tag='xt', bufs=16,
                                   name=f'xt{j}_{k}')
                    nc.sync.dma_start(
                        out=xk[:], in_=xt[k * 128:(k + 1) * 128, ts:ts + 512])
                    xt_t.append(xk)
                for dst in (2, 0, 1):          # v first: its DVE copy + PE
                    d_ps = ps.tile([128, 512], f32, tag='qkv', bufs=2,
                                   name=f'd{j}_{dst}')
                    for k in range(NE):
                        nc.tensor.matmul(
                            d_ps[:],
                            wqkv_sb[k][:, dst * 128:(dst + 1) * 128],
                            xt_t[k][:], start=(k == 0), stop=(k == NE - 1))
                    if dst == 0:
                        nc.vector.tensor_copy(qT_sb[j][0][0:64, :],
                                              d_ps[0:64, :])
                        nc.scalar.copy(qT_sb[j][1][0:64, :], d_ps[64:128, :])
                    elif dst == 1:
                        nc.vector.tensor_copy(kT_sb[j][0][0:64, :],
                                              d_ps[0:64, :])
                        nc.vector.tensor_copy(kT_sb[j][1][0:64, :],
                                              d_ps[64:128, :])
                    else:
                        vt_sb = work.tile([128, 512], bf16, tag='vt', bufs=2)
                        nc.vector.tensor_copy(vt_sb[:], d_ps[:])
                        for sc in range(4):
                            vst = work.tile([128, 128], bf16, tag='vst',
                                            bufs=4, name=f'vst{j}_{sc}')
                            nc.sync.dma_start_transpose(
                                out=vst[:],
                                in_=vt_sb[:, sc * 128:(sc + 1) * 128])
                            nc.vector.tensor_copy(
                                v_sb[j * 4 + sc][:, :, 0:64],
                                vst.rearrange('p (h e) -> p h e', h=2))

            # ---- B(j): scores + AV accumulation ----
            def emit_B(j):
                b, tb = divmod(j, NTB)
                n_si = 4 * (tb + 1)
                av_pss = [ps.tile([65, 512], f32, tag='av', bufs=4,
                                  name=f'av{j}_{h}') for h in range(HPC)]
                st[j]['av'] = av_pss
                # software-pipelined: AV matmuls trail their scores by LAG
                # groups so the PE never waits on the exp chain
                LAG = 6
                groups = [(si, h) for si in range(n_si) for h in range(HPC)]
                pend = []

                def emit_score(si, h):
                    sblk = b * NTB + si // 4
                    srem = (si % 4) * 128
                    woff = (si - 4 * tb) * 128 if si >= 4 * tb else 0
                    w_ps = ps.tile([128, 512], f32, tag='wei', bufs=2,
                                   name=f'w{j}_{si}_{h}')
                    nc.tensor.matmul(
                        w_ps[:, woff:512],
                        kT_sb[sblk][h][:, srem:srem + 128],
                        qT_sb[j][h][:, woff:512],
                        start=True, stop=True)
                    wt = work.tile([128, 512], bf16, tag='wt', bufs=24,
                                   name=f'wt{j}_{si}_{h}')
                    nc.scalar.activation(wt[:, woff:512], w_ps[:, woff:512],
                                         EXP, scale=SCALE)
                    if si >= 4 * tb:
                        nc.vector.tensor_mul(wt[:, woff:woff + 128],
                                             wt[:, woff:woff + 128],
                                             tmask[:])
                    return (si, h, woff, wt)

                def emit_av(si, h, woff, wt):
                    nc.tensor.matmul(
                        av_pss[h][:, woff:512],
                        v_sb[b * NST + si][:, h, :],
                        wt[:, woff:512],
                        start=(si == 0), stop=(si == n_si - 1),
                        skip_group_check=True)

                for idx, (si, h) in enumerate(groups):
                    pend.append(emit_score(si, h))
                    if idx >= LAG:
                        emit_av(*pend.pop(0))
                for p in pend:
                    emit_av(*p)

            # ---- L(j): stash softmax denominator rows ----
            def emit_L(j):
                lrow = lrow_t[j % 2]
                st[j]['lrow'] = lrow
                for h in range(HPC):
                    nc.vector.tensor_copy(lrow[32 * h:32 * h + 1, :],
                                          st[j]['av'][h][64:65, :])

            # ---- M(j): broadcast l, reciprocal, normalize ----
            def emit_M(j):
                bc_ps = ps.tile([128, 512], f32, tag='qkv', bufs=2,
                                name=f'bc{j}')
                nc.tensor.matmul(bc_ps[:], sel_bc[:], st[j]['lrow'][:],
                                 start=True, stop=True)
                rc_sb = work.tile([128, 512], f32, tag='rc', bufs=2)
                nc.vector.reciprocal(rc_sb[:], bc_ps[:])
                avT_sb = work.tile([128, 512], bf16, tag='avT', bufs=2)
                st[j]['avT'] = avT_sb
                for h in range(HPC):
                    hd = h * 64
                    nc.vector.tensor_mul(avT_sb[hd:hd + 64, :],
                                         st[j]['av'][h][0:64, :],
                                         rc_sb[hd:hd + 64, :])

            # ---- P(j): output projection + DMA ----
            def emit_P(j, final=False):
                t0 = j * 512
                avT_sb = st[j]['avT']
                for tc4 in range(4):
                    for eb in range(2):
                        idx = tc4 * 2 + eb
                        y_ps = ps.tile([128, 512], f32, tag='wei', bufs=2,
                                       name=f'y{j}_{tc4}_{eb}')
                        nc.tensor.matmul(
                            y_ps[:],
                            avT_sb[:, tc4 * 128:(tc4 + 1) * 128],
                            wproj_sb[:, eb * 512:(eb + 1) * 512],
                            start=True, stop=True)
                        y_sb = work.tile([128, 512], f32, tag='ysb', bufs=6)
                        if idx % 4 == 1:
                            nc.scalar.copy(y_sb[:], y_ps[:])
                        else:
                            nc.vector.tensor_copy(y_sb[:], y_ps[:])
                        nc.sync.dma_start(
                            out=y[t0 + tc4 * 128:t0 + (tc4 + 1) * 128,
                                  eb * 512:(eb + 1) * 512],
                            in_=y_sb[:])

            # ---- fused final epilogue: pipeline normalize+proj+copy+DMA
            # per 128-t quarter so the serial tail shrinks
            def emit_MP_final(j):
                t0 = j * 512
                bc_ps = ps.tile([128, 512], f32, tag='qkv', bufs=2,
                                name=f'bcf{j}')
                nc.tensor.matmul(bc_ps[:], sel_bc[:], st[j]['lrow'][:],
                                 start=True, stop=True)
                rc_sb = work.tile([128, 512], f32, tag='rc', bufs=2)
                avT_sb = work.tile([128, 512], bf16, tag='avT', bufs=2)
                for tc4 in range(4):
                    cs = slice(tc4 * 128, (tc4 + 1) * 128)
                    nc.vector.reciprocal(rc_sb[:, cs], bc_ps[:, cs])
                    for h in range(HPC):
                        hd = h * 64
                        nc.vector.tensor_mul(avT_sb[hd:hd + 64, cs],
                                             st[j]['av'][h][0:64, cs],
                                             rc_sb[hd:hd + 64, cs])
                    for eb in range(2):
                        y_ps = ps.tile([128, 512], f32, tag='wei', bufs=2,
                                       name=f'yf{j}_{tc4}_{eb}')
                        nc.tensor.matmul(
                            y_ps[:], avT_sb[:, cs],
                            wproj_sb[:, eb * 512:(eb + 1) * 512],
                            start=True, stop=True)
                        y_sb = work.tile([128, 512], f32, tag='ysb', bufs=6)
                        if eb == 0:
                            nc.vector.tensor_copy(y_sb[:], y_ps[:])
                        else:
                            nc.scalar.copy(y_sb[:], y_ps[:])
                        nc.sync.dma_start(
                            out=y[t0 + tc4 * 128:t0 + (tc4 + 1) * 128,
                                  eb * 512:(eb + 1) * 512],
                            in_=y_sb[:])

            # ---- software-pipelined schedule ----
            # B-order puts the smallest attention block (b1,t0 = j4) last so
            # the un-hideable softmax/proj tail follows a short B phase
            border = [0, 1, 2, 3, 5, 6, 7, 4]
            prev = None
            nextA = 0
            for j in border:
                while nextA <= j:
                    emit_A(nextA)
                    nextA += 1
                if prev is not None:
                    emit_M(prev)
                emit_B(j)
                if prev is not None:
                    emit_P(prev)
                emit_L(j)
                prev = j
            emit_MP_final(prev)

    import concourse.mybir as mybir2
    _split_multi_waits(nc, mybir2)
    return nc


_CACHE = {}


def kernel(x, Wq, Wk, Wv, Wproj, bproj):
    _install_ntff_hook()
    import ml_dtypes
    from concourse.bass_utils import run_bass_kernel_spmd

    bf = ml_dtypes.bfloat16
    x = np.asarray(x, dtype=np.float32)
    Wq = np.asarray(Wq, dtype=np.float32)
    Wk = np.asarray(Wk, dtype=np.float32)
    Wv = np.asarray(Wv, dtype=np.float32)
    Wproj = np.asarray(Wproj, dtype=np.float32)
    bproj = np.asarray(bproj, dtype=np.float32)

    if 'nc' not in _CACHE:
        _CACHE['nc'] = _build_nc()
    nc = _CACHE['nc']

    xT = np.ascontiguousarray(x.reshape(BT, E).T).astype(bf)
    in_maps = []
    for c in range(N_CORES):
        h0 = HPC * c
        wqkv_c = np.concatenate(
            [Wq[h0], Wq[h0 + 1], Wk[h0], Wk[h0 + 1], Wv[h0], Wv[h0 + 1]],
            axis=1)                                         # [E, 384]
        wproj_c = np.ascontiguousarray(Wproj[DPC * c: DPC * (c + 1)])
        in_maps.append({'xt': xT,
                        'wqkv': np.ascontiguousarray(wqkv_c).astype(bf),
                        'wproj': wproj_c.astype(bf)})

    res = run_bass_kernel_spmd(nc, in_maps, list(range(N_CORES)))
    ysum = np.zeros((BT, E), dtype=np.float64)
    for c in range(N_CORES):
        ysum += res.results[c]['y'].astype(np.float64)
    out = (ysum + bproj.astype(np.float64)).astype(np.float32)
    return out.reshape(B, T, E)


# revision 33
# speedup vs baseline: 1.0648x; 1.0648x over previous
"""Multi-head causal attention (B=2, T=2048, E=1024, H=16, D=64) on 8 TRN2
NeuronCores, tensor-parallel over heads (2 heads per core).

v2: all-bf16 matmuls (PSUM accum f32), causal column-trimmed score/AV
matmuls, and a software-pipelined emission schedule so the PE queue never
stalls on the softmax epilogue:

  per 512-t block j:  A(j) qkv projection -> B(j) scores+AV -> L(j) stash
  denominator rows -> M(j) broadcast l via PE + reciprocal + normalize ->
  P(j) out-proj + DMA.
  Emission order: A(j), M(j-1), B(j), P(j-1), L(j)  -- the PE executes
  A/B matmuls of block j while the DVE reciprocal chain of block j-1 runs.

Dataflow per core: xt = x^T [E, BT] bf16 (same on all cores); wqkv_c
[E, 384] (2 heads of Wq|Wk|Wv); wproj_c = Wproj[128c:128c+128, :].
  A: qT,kT,vT [d,t] = wqkv_c^T @ xt (dest-major, PSUM accum over E tiles);
     v[s,d] via PE transpose, with a ones column per head appended so the
     softmax denominator falls out of the AV matmul (row 64).
  B: per 128-s-tile x head: weiT[s,t] = kT^T q on cols [woff:512] only
     (causal), Exp on ACT (scale=E^-0.5) PSUM->SBUF bf16, multiplicative
     0/1 tril mask on the diagonal chunk, avT_aug[65,t] += [v|1]^T wt.
  M: bc[128,t] = sel^T @ l rows (PE broadcast, f32r), 1/bc on DVE
     (doubles as PSUM->SBUF move), two muls -> merged-head proj lhsT.
  P: y[t,e] = avT^T @ wproj per 128-t chunk; PSUM->SBUF copy alternates
     DVE/GpSimd; DMA out. host: y = sum_c y_c + bproj.

Infra notes: this container's walrus accepts at most ONE semaphore wait
per instruction (_split_multi_waits hoists extras onto EventSemaphores);
the ACT engine must run a single function (Exp) to avoid ~1.3us
activation-table swaps. PSUM rings: qkv(2) + wei(2) + av(4) = 8 banks.
"""
import sys
import types

import numpy as np

B, T, E, H, D = 2, 2048, 1024, 16, 64
N_CORES = 8
HPC = H // N_CORES          # heads per core = 2
BT = B * T                  # 4096
DPC = HPC * D               # 128 head-dims per core
SCALE = 1.0 / float(np.sqrt(E))  # NOTE: reference scales by E**-0.5
NB = BT // 512              # 8 global 512-t blocks
NTB = T // 512              # 4 t-blocks per batch
NST = T // 128              # 16 s-tiles per batch
NE = E // 128               # 8 e-tiles


def _install_ntff_hook():
    if 'antenv.axon_hooks' in sys.modules:
        return
    try:
        sys.path.insert(0, '/root/.axon_site')
        from trn_agent_boot.trn_boot import _ntff_profile_via_ctypes
        hook = _ntff_profile_via_ctypes('/opt/axon/libaxon_pjrt.so')
        mod = types.ModuleType('antenv.axon_hooks')
        mod.get_axon_ntff_profile_hook = lambda: hook
        mod.set_axon_ntff_profile_hook = lambda h: None
        sys.modules['antenv.axon_hooks'] = mod
    except Exception:
        pass


def _split_multi_waits(nc, mybir):
    """This walrus build rejects >1 sync-wait per instruction. Hoist extra
    waits onto EventSemaphore instructions on the same engine just before."""
    for f in nc.m.functions:
        for bb in f.blocks:
            new_insts = []
            changed = False
            for inst in bb.instructions:
                si = inst.sync_info
                if si is not None and len(si.on_wait) > 1:
                    extra = list(si.on_wait[:-1])
                    keep = si.on_wait[-1]
                    for w in extra:
                        ev = mybir.InstEventSemaphore(
                            name=f"I-{nc.next_id()}", ins=[], outs=[])
                        ev.engine = inst.engine
                        ev.sync_info = mybir.SyncInfo(on_wait=[w], on_update=[])
                        new_insts.append(ev)
                    del si.on_wait[:]
                    si.on_wait.append(keep)
                    changed = True
                new_insts.append(inst)
            if changed:
                bb.instructions = new_insts


def _build_nc():
    import concourse.bass as bass
    import concourse.mybir as mybir
    import concourse.tile as tile
    from concourse.masks import make_identity

    f32 = mybir.dt.float32
    f32r = mybir.dt.float32r
    bf16 = mybir.dt.bfloat16
    EXP = mybir.ActivationFunctionType.Exp

    nc = bass.Bass('TRN2', num_devices=N_CORES)
    xt = nc.dram_tensor('xt', [E, BT], bf16, kind='ExternalInput')
    wqkv = nc.dram_tensor('wqkv', [E, 3 * DPC], bf16, kind='ExternalInput')
    wproj = nc.dram_tensor('wproj', [DPC, E], bf16, kind='ExternalInput')
    y = nc.dram_tensor('y', [BT, E], f32, kind='ExternalOutput')

    with tile.TileContext(nc) as tc:
        with tc.tile_pool(name='consts', bufs=1) as consts, \
             tc.tile_pool(name='big', bufs=1) as big, \
             tc.tile_pool(name='work', bufs=1) as work, \
             tc.tile_pool(name='ps', bufs=1, space='PSUM') as ps:

            # ---- constants ----
            ident_f = consts.tile([128, 128], f32)
            make_identity(nc, ident_f)
            ident = consts.tile([128, 128], bf16)
            nc.vector.tensor_copy(ident[:], ident_f[:])
            # multiplicative mask for the diagonal chunk of weiT [s,t]:
            # keep (1) where t >= s, 0 where t < s
            tmask_f = consts.tile([128, 128], f32)
            nc.gpsimd.memset(tmask_f[:], 1.0)
            nc.gpsimd.affine_select(
                out=tmask_f[:], in_=tmask_f[:],
                compare_op=mybir.AluOpType.is_ge,
                fill=0.0, base=0, pattern=[[1, 128]], channel_multiplier=-1)
            tmask = consts.tile([128, 128], bf16)
            nc.vector.tensor_copy(tmask[:], tmask_f[:])
            # sel [33,128]: row0 -> partitions 0-63 (head0), row32 -> 64-127
            sel_f32 = consts.tile([33, 128], f32)
            nc.gpsimd.memset(sel_f32[:], 0.0)
            nc.gpsimd.memset(sel_f32[0:1, 0:64], 1.0)
            nc.gpsimd.memset(sel_f32[32:33, 64:128], 1.0)
            sel_bc = consts.tile([33, 128], f32r)   # lhsT of bcast matmul
            nc.vector.tensor_copy(sel_bc[:], sel_f32[:])
            # manually double-buffered denominator rows (only rows 0/32 are
            # ever rewritten; the rest must stay finite for the bcast matmul)
            lrow_t = [consts.tile([33, 512], f32r, name=f'lrow{i}')
                      for i in range(2)]
            for i in range(2):
                nc.gpsimd.memset(lrow_t[i][:].bitcast(mybir.dt.uint32),
                                 1065353216)

            # ---- weights ----
            wqkv_sb = [consts.tile([128, 3 * DPC], bf16, name=f'wqkv{k}')
                       for k in range(NE)]
            dma_qs = [nc.sync, nc.gpsimd, nc.scalar]
            for k in range(NE):
                dma_qs[k % 3].dma_start(out=wqkv_sb[k][:],
                                        in_=wqkv[k * 128:(k + 1) * 128, :])
            wproj_sb = consts.tile([DPC, E], bf16)
            nc.gpsimd.dma_start(out=wproj_sb[:], in_=wproj[:])

            # ---- persistent activations ----
            # per-head q/k tiles re-based to partition 0 with rows 64-127
            # zeroed once: score matmuls run K=128 (full-rate PE tile mode,
            # the zero pad rows contribute nothing)
            qT_sb = [[big.tile([128, 512], bf16, name=f'q{j}_{h}')
                      for h in range(HPC)] for j in range(NB)]
            kT_sb = [[big.tile([128, 512], bf16, name=f'k{j}_{h}')
                      for h in range(HPC)] for j in range(NB)]

            # v tiles [s,d] per 128-s-tile, layout [128, 2, 65]: per head 64
            # dims + ones column (softmax denominator via the AV matmul)
            v_sb = [big.tile([128, 2, 65], bf16, name=f'v{si}')
                    for si in range(2 * NST)]

            # per-block state carried between emission stages
            st = [dict() for _ in range(NB)]

            # ---- A(j): QKV projection for one 512-t block ----
            def emit_A(j):
                ts = j * 512
                # one-time zero/ones fills for this block's pad tiles,
                # spread across the run to keep the startup short
                for h in range(HPC):
                    nc.gpsimd.memset(
                        qT_sb[j][h][64:128, :].bitcast(mybir.dt.uint16), 0)
                    nc.gpsimd.memset(
                        kT_sb[j][h][64:128, :].bitcast(mybir.dt.uint16), 0)
                for sc in range(4):
                    nc.gpsimd.memset(
                        v_sb[j * 4 + sc][:, :, 64:65].bitcast(
                            mybir.dt.uint16), 16256)
                xt_t = []
                for k in range(NE):
                    xk = work.tile([128, 512], bf16, tag='xt', bufs=16,
                                   name=f'xt{j}_{k}')
                    nc.sync.dma_start(
                        out=xk[:], in_=xt[k * 128:(k + 1) * 128, ts:ts + 512])
                    xt_t.append(xk)
                for dst in (2, 0, 1):          # v first: its DVE copy + PE
                    d_ps = ps.tile([128, 512], f32, tag='qkv', bufs=2,
                                   name=f'd{j}_{dst}')
                    for k in range(NE):
                        nc.tensor.matmul(
                            d_ps[:],
                            wqkv_sb[k][:, dst * 128:(dst + 1) * 128],
                            xt_t[k][:], start=(k == 0), stop=(k == NE - 1))
                    if dst == 0:
                        nc.vector.tensor_copy(qT_sb[j][0][0:64, :],
                                              d_ps[0:64, :])
                        nc.scalar.copy(qT_sb[j][1][0:64, :], d_ps[64:128, :])
                    elif dst == 1:
                        nc.vector.tensor_copy(kT_sb[j][0][0:64, :],
                                              d_ps[0:64, :])
                        nc.vector.tensor_copy(kT_sb[j][1][0:64, :],
                                              d_ps[64:128, :])
                    else:
                        vt_sb = work.tile([128, 512], bf16, tag='vt', bufs=2)
                        nc.vector.tensor_copy(vt_sb[:], d_ps[:])
                        for sc in range(4):
                            vtr = ps.tile([128, 128], bf16, tag='wei',
                                          bufs=2, name=f'vtr{j}_{sc}')
                            nc.tensor.transpose(
                                vtr[:], vt_sb[:, sc * 128:(sc + 1) * 128],
                                ident[:])
                            nc.vector.tensor_copy(
                                v_sb[j * 4 + sc][:, :, 0:64],
                                vtr.rearrange('p (h e) -> p h e', h=2))

            # ---- B(j): scores + AV accumulation ----
            def emit_B(j):
                b, tb = divmod(j, NTB)
                n_si = 4 * (tb + 1)
                av_pss = [ps.tile([65, 512], f32, tag='av', bufs=4,
                                  name=f'av{j}_{h}') for h in range(HPC)]
                st[j]['av'] = av_pss
                # software-pipelined: AV matmuls trail their scores by LAG
                # groups so the PE never waits on the exp chain
                LAG = 6
                groups = [(si, h) for si in range(n_si) for h in range(HPC)]
                pend = []

                def emit_score(si, h):
                    sblk = b * NTB + si // 4
                    srem = (si % 4) * 128
                    woff = (si - 4 * tb) * 128 if si >= 4 * tb else 0
                    w_ps = ps.tile([128, 512], f32, tag='wei', bufs=2,
                                   name=f'w{j}_{si}_{h}')
                    nc.tensor.matmul(
                        w_ps[:, woff:512],
                        kT_sb[sblk][h][:, srem:srem + 128],
                        qT_sb[j][h][:, woff:512],
                        start=True, stop=True)
                    wt = work.tile([128, 512], bf16, tag='wt', bufs=24,
                                   name=f'wt{j}_{si}_{h}')
                    nc.scalar.activation(wt[:, woff:512], w_ps[:, woff:512],
                                         EXP, scale=SCALE)
                    if si >= 4 * tb:
                        nc.vector.tensor_mul(wt[:, woff:woff + 128],
                                             wt[:, woff:woff + 128],
                                             tmask[:])
                    return (si, h, woff, wt)

                def emit_av(si, h, woff, wt):
                    nc.tensor.matmul(
                        av_pss[h][:, woff:512],
                        v_sb[b * NST + si][:, h, :],
                        wt[:, woff:512],
                        start=(si == 0), stop=(si == n_si - 1),
                        skip_group_check=True)

                for idx, (si, h) in enumerate(groups):
                    pend.append(emit_score(si, h))
                    if idx >= LAG:
                        emit_av(*pend.pop(0))
                for p in pend:
                    emit_av(*p)

            # ---- L(j): stash softmax denominator rows ----
            def emit_L(j):
                lrow = lrow_t[j % 2]
                st[j]['lrow'] = lrow
                for h in range(HPC):
                    nc.vector.tensor_copy(lrow[32 * h:32 * h + 1, :],
                                          st[j]['av'][h][64:65, :])

            # ---- M(j): broadcast l, reciprocal, normalize ----
            def emit_M(j):
                bc_ps = ps.tile([128, 512], f32, tag='qkv', bufs=2,
                                name=f'bc{j}')
                nc.tensor.matmul(bc_ps[:], sel_bc[:], st[j]['lrow'][:],
                                 start=True, stop=True)
                rc_sb = work.tile([128, 512], f32, tag='rc', bufs=2)
                nc.vector.reciprocal(rc_sb[:], bc_ps[:])
                avT_sb = work.tile([128, 512], bf16, tag='avT', bufs=2)
                st[j]['avT'] = avT_sb
                for h in range(HPC):
                    hd = h * 64
                    nc.vector.tensor_mul(avT_sb[hd:hd + 64, :],
                                         st[j]['av'][h][0:64, :],
                                         rc_sb[hd:hd + 64, :])

            # ---- P(j): output projection + DMA ----
            def emit_P(j, final=False):
                t0 = j * 512
                avT_sb = st[j]['avT']
                for tc4 in range(4):
                    for eb in range(2):
                        idx = tc4 * 2 + eb
                        y_ps = ps.tile([128, 512], f32, tag='wei', bufs=2,
                                       name=f'y{j}_{tc4}_{eb}')
                        nc.tensor.matmul(
                            y_ps[:],
                            avT_sb[:, tc4 * 128:(tc4 + 1) * 128],
                            wproj_sb[:, eb * 512:(eb + 1) * 512],
                            start=True, stop=True)
                        y_sb = work.tile([128, 512], f32, tag='ysb', bufs=6)
                        if idx % 4 == 1:
                            nc.scalar.copy(y_sb[:], y_ps[:])
                        else:
                            nc.vector.tensor_copy(y_sb[:], y_ps[:])
                        nc.sync.dma_start(
                            out=y[t0 + tc4 * 128:t0 + (tc4 + 1) * 128,
                                  eb * 512:(eb + 1) * 512],
                            in_=y_sb[:])

            # ---- fused final epilogue: pipeline normalize+proj+copy+DMA
            # per 128-t quarter so the serial tail shrinks
            def emit_MP_final(j):
                t0 = j * 512
                bc_ps = ps.tile([128, 512], f32, tag='qkv', bufs=2,
                                name=f'bcf{j}')
                nc.tensor.matmul(bc_ps[:], sel_bc[:], st[j]['lrow'][:],
                                 start=True, stop=True)
                rc_sb = work.tile([128, 512], f32, tag='rc', bufs=2)
                avT_sb = work.tile([128, 512], bf16, tag='avT', bufs=2)
                for tc4 in range(4):
                    cs = slice(tc4 * 128, (tc4 + 1) * 128)
                    nc.vector.reciprocal(rc_sb[:, cs], bc_ps[:, cs])
                    for h in range(HPC):
                        hd = h * 64
                        nc.vector.tensor_mul(avT_sb[hd:hd + 64, cs],
                                             st[j]['av'][h][0:64, cs],
                                             rc_sb[hd:hd + 64, cs])
                    for eb in range(2):
                        y_ps = ps.tile([128, 512], f32, tag='wei', bufs=2,
                                       name=f'yf{j}_{tc4}_{eb}')
                        nc.tensor.matmul(
                            y_ps[:], avT_sb[:, cs],
                            wproj_sb[:, eb * 512:(eb + 1) * 512],
                            start=True, stop=True)
                        y_sb = work.tile([128, 512], f32, tag='ysb', bufs=6)
                        if eb == 0:
                            nc.vector.tensor_copy(y_sb[:], y_ps[:])
                        else:
                            nc.scalar.copy(y_sb[:], y_ps[:])
                        nc.sync.dma_start(
                            out=y[t0 + tc4 * 128:t0 + (tc4 + 1) * 128,
                                  eb * 512:(eb + 1) * 512],
                            in_=y_sb[:])

            # ---- software-pipelined schedule ----
            # B-order puts the smallest attention block (b1,t0 = j4) last so
            # the un-hideable softmax/proj tail follows a short B phase
            border = [0, 1, 2, 3, 5, 6, 7, 4]
            prev = None
            nextA = 0
            for j in border:
                while nextA <= j:
                    emit_A(nextA)
                    nextA += 1
                if prev is not None:
                    emit_M(prev)
                emit_B(j)
                if prev is not None:
                    emit_P(prev)
                emit_L(j)
                prev = j
            emit_MP_final(prev)

    import concourse.mybir as mybir2
    _split_multi_waits(nc, mybir2)
    return nc


_CACHE = {}


def kernel(x, Wq, Wk, Wv, Wproj, bproj):
    _install_ntff_hook()
    import ml_dtypes
    from concourse.bass_utils import run_bass_kernel_spmd

    bf = ml_dtypes.bfloat16
    x = np.asarray(x, dtype=np.float32)
    Wq = np.asarray(Wq, dtype=np.float32)
    Wk = np.asarray(Wk, dtype=np.float32)
    Wv = np.asarray(Wv, dtype=np.float32)
    Wproj = np.asarray(Wproj, dtype=np.float32)
    bproj = np.asarray(bproj, dtype=np.float32)

    if 'nc' not in _CACHE:
        _CACHE['nc'] = _build_nc()
    nc = _CACHE['nc']

    xT = np.ascontiguousarray(x.reshape(BT, E).T).astype(bf)
    in_maps = []
    for c in range(N_CORES):
        h0 = HPC * c
        wqkv_c = np.concatenate(
            [Wq[h0], Wq[h0 + 1], Wk[h0], Wk[h0 + 1], Wv[h0], Wv[h0 + 1]],
            axis=1)                                         # [E, 384]
        wproj_c = np.ascontiguousarray(Wproj[DPC * c: DPC * (c + 1)])
        in_maps.append({'xt': xT,
                        'wqkv': np.ascontiguousarray(wqkv_c).astype(bf),
                        'wproj': wproj_c.astype(bf)})

    res = run_bass_kernel_spmd(nc, in_maps, list(range(N_CORES)))
    ysum = np.zeros((BT, E), dtype=np.float64)
    for c in range(N_CORES):
        ysum += res.results[c]['y'].astype(np.float64)
    out = (ysum + bproj.astype(np.float64)).astype(np.float32)
    return out.reshape(B, T, E)


# revision 34
# speedup vs baseline: 1.1148x; 1.0469x over previous
"""Multi-head causal attention (B=2, T=2048, E=1024, H=16, D=64) on 8 TRN2
NeuronCores, tensor-parallel over heads (2 heads per core).

v2: all-bf16 matmuls (PSUM accum f32), causal column-trimmed score/AV
matmuls, and a software-pipelined emission schedule so the PE queue never
stalls on the softmax epilogue:

  per 512-t block j:  A(j) qkv projection -> B(j) scores+AV -> L(j) stash
  denominator rows -> M(j) broadcast l via PE + reciprocal + normalize ->
  P(j) out-proj + DMA.
  Emission order: A(j), M(j-1), B(j), P(j-1), L(j)  -- the PE executes
  A/B matmuls of block j while the DVE reciprocal chain of block j-1 runs.

Dataflow per core: xt = x^T [E, BT] bf16 (same on all cores); wqkv_c
[E, 384] (2 heads of Wq|Wk|Wv); wproj_c = Wproj[128c:128c+128, :].
  A: qT,kT,vT [d,t] = wqkv_c^T @ xt (dest-major, PSUM accum over E tiles);
     v[s,d] via PE transpose, with a ones column per head appended so the
     softmax denominator falls out of the AV matmul (row 64).
  B: per 128-s-tile x head: weiT[s,t] = kT^T q on cols [woff:512] only
     (causal), Exp on ACT (scale=E^-0.5) PSUM->SBUF bf16, multiplicative
     0/1 tril mask on the diagonal chunk, avT_aug[65,t] += [v|1]^T wt.
  M: bc[128,t] = sel^T @ l rows (PE broadcast, f32r), 1/bc on DVE
     (doubles as PSUM->SBUF move), two muls -> merged-head proj lhsT.
  P: y[t,e] = avT^T @ wproj per 128-t chunk; PSUM->SBUF copy alternates
     DVE/GpSimd; DMA out. host: y = sum_c y_c + bproj.

Infra notes: this container's walrus accepts at most ONE semaphore wait
per instruction (_split_multi_waits hoists extras onto EventSemaphores);
the ACT engine must run a single function (Exp) to avoid ~1.3us
activation-table swaps. PSUM rings: qkv(2) + wei(2) + av(4) = 8 banks.
"""
import sys
import types

import numpy as np

B, T, E, H, D = 2, 2048, 1024, 16, 64
N_CORES = 8
HPC = H // N_CORES          # heads per core = 2
BT = B * T                  # 4096
DPC = HPC * D               # 128 head-dims per core
SCALE = 1.0 / float(np.sqrt(E))  # NOTE: reference scales by E**-0.5
NB = BT // 512              # 8 global 512-t blocks
NTB = T // 512              # 4 t-blocks per batch
NST = T // 128              # 16 s-tiles per batch
NE = E // 128               # 8 e-tiles


def _install_ntff_hook():
    if 'antenv.axon_hooks' in sys.modules:
        return
    try:
        sys.path.insert(0, '/root/.axon_site')
        from trn_agent_boot.trn_boot import _ntff_profile_via_ctypes
        hook = _ntff_profile_via_ctypes('/opt/axon/libaxon_pjrt.so')
        mod = types.ModuleType('antenv.axon_hooks')
        mod.get_axon_ntff_profile_hook = lambda: hook
        mod.set_axon_ntff_profile_hook = lambda h: None
        sys.modules['antenv.axon_hooks'] = mod
    except Exception:
        pass


def _split_multi_waits(nc, mybir):
    """This walrus build rejects >1 sync-wait per instruction. Hoist extra
    waits onto EventSemaphore instructions on the same engine just before."""
    for f in nc.m.functions:
        for bb in f.blocks:
            new_insts = []
            changed = False
            for inst in bb.instructions:
                si = inst.sync_info
                if si is not None and len(si.on_wait) > 1:
                    extra = list(si.on_wait[:-1])
                    keep = si.on_wait[-1]
                    for w in extra:
                        ev = mybir.InstEventSemaphore(
                            name=f"I-{nc.next_id()}", ins=[], outs=[])
                        ev.engine = inst.engine
                        ev.sync_info = mybir.SyncInfo(on_wait=[w], on_update=[])
                        new_insts.append(ev)
                    del si.on_wait[:]
                    si.on_wait.append(keep)
                    changed = True
                new_insts.append(inst)
            if changed:
                bb.instructions = new_insts


def _build_nc():
    import concourse.bass as bass
    import concourse.mybir as mybir
    import concourse.tile as tile
    from concourse.masks import make_identity

    f32 = mybir.dt.float32
    f32r = mybir.dt.float32r
    bf16 = mybir.dt.bfloat16
    EXP = mybir.ActivationFunctionType.Exp

    nc = bass.Bass('TRN2', num_devices=N_CORES)
    xt = nc.dram_tensor('xt', [E, BT], bf16, kind='ExternalInput')
    wqkv = nc.dram_tensor('wqkv', [E, 3 * DPC], bf16, kind='ExternalInput')
    wproj = nc.dram_tensor('wproj', [DPC, E], bf16, kind='ExternalInput')
    y = nc.dram_tensor('y', [BT, E], f32, kind='ExternalOutput')

    with tile.TileContext(nc) as tc:
        with tc.tile_pool(name='consts', bufs=1) as consts, \
             tc.tile_pool(name='big', bufs=1) as big, \
             tc.tile_pool(name='work', bufs=1) as work, \
             tc.tile_pool(name='ps', bufs=1, space='PSUM') as ps:

            # ---- constants ----
            ident_f = consts.tile([128, 128], f32)
            make_identity(nc, ident_f)
            ident = consts.tile([128, 128], bf16)
            nc.vector.tensor_copy(ident[:], ident_f[:])
            # multiplicative mask for the diagonal chunk of weiT [s,t]:
            # keep (1) where t >= s, 0 where t < s
            tmask_f = consts.tile([128, 128], f32)
            nc.gpsimd.memset(tmask_f[:], 1.0)
            nc.gpsimd.affine_select(
                out=tmask_f[:], in_=tmask_f[:],
                compare_op=mybir.AluOpType.is_ge,
                fill=0.0, base=0, pattern=[[1, 128]], channel_multiplier=-1)
            tmask = consts.tile([128, 128], bf16)
            nc.vector.tensor_copy(tmask[:], tmask_f[:])
            # sel [33,128]: row0 -> partitions 0-63 (head0), row32 -> 64-127
            sel_f32 = consts.tile([33, 128], f32)
            nc.gpsimd.memset(sel_f32[:], 0.0)
            nc.gpsimd.memset(sel_f32[0:1, 0:64], 1.0)
            nc.gpsimd.memset(sel_f32[32:33, 64:128], 1.0)
            sel_bc = consts.tile([33, 128], f32r)   # lhsT of bcast matmul
            nc.vector.tensor_copy(sel_bc[:], sel_f32[:])
            # manually double-buffered denominator rows (only rows 0/32 are
            # ever rewritten; the rest must stay finite for the bcast matmul)
            lrow_t = [consts.tile([33, 512], f32r, name=f'lrow{i}')
                      for i in range(2)]
            for i in range(2):
                nc.gpsimd.memset(lrow_t[i][:].bitcast(mybir.dt.uint32),
                                 1065353216)

            # ---- weights ----
            wqkv_sb = [consts.tile([128, 3 * DPC], bf16, name=f'wqkv{k}')
                       for k in range(NE)]
            dma_qs = [nc.sync, nc.gpsimd, nc.scalar]
            for k in range(NE):
                dma_qs[k % 3].dma_start(out=wqkv_sb[k][:],
                                        in_=wqkv[k * 128:(k + 1) * 128, :])
            wproj_sb = consts.tile([DPC, E], bf16)
            nc.gpsimd.dma_start(out=wproj_sb[:], in_=wproj[:])

            # ---- persistent activations ----
            # per-head q/k tiles re-based to partition 0 with rows 64-127
            # zeroed once: score matmuls run K=128 (full-rate PE tile mode,
            # the zero pad rows contribute nothing)
            qT_sb = [[big.tile([128, 512], bf16, name=f'q{j}_{h}')
                      for h in range(HPC)] for j in range(NB)]
            kT_sb = [[big.tile([128, 512], bf16, name=f'k{j}_{h}')
                      for h in range(HPC)] for j in range(NB)]

            # v tiles [s,d] per 128-s-tile, layout [128, 2, 65]: per head 64
            # dims + ones column (softmax denominator via the AV matmul)
            v_sb = [big.tile([128, 2, 65], bf16, name=f'v{si}')
                    for si in range(2 * NST)]

            # per-block state carried between emission stages
            st = [dict() for _ in range(NB)]

            # ---- A(j): QKV projection for one 512-t block ----
            def emit_A(j):
                ts = j * 512
                # one-time zero/ones fills for this block's pad tiles,
                # spread across the run to keep the startup short
                for h in range(HPC):
                    nc.gpsimd.memset(
                        qT_sb[j][h][64:128, :].bitcast(mybir.dt.uint16), 0)
                    nc.gpsimd.memset(
                        kT_sb[j][h][64:128, :].bitcast(mybir.dt.uint16), 0)
                for sc in range(4):
                    nc.gpsimd.memset(
                        v_sb[j * 4 + sc][:, :, 64:65].bitcast(
                            mybir.dt.uint16), 16256)
                xt_t = []
                for k in range(NE):
                    xk = work.tile([128, 512], bf16, tag='xt', bufs=16,
                                   name=f'xt{j}_{k}')
                    nc.sync.dma_start(
                        out=xk[:], in_=xt[k * 128:(k + 1) * 128, ts:ts + 512])
                    xt_t.append(xk)
                for dst in (2, 0, 1):          # v first: its DVE copy + PE
                    d_ps = ps.tile([128, 512], f32, tag='qkv', bufs=2,
                                   name=f'd{j}_{dst}')
                    for k in range(NE):
                        nc.tensor.matmul(
                            d_ps[:],
                            wqkv_sb[k][:, dst * 128:(dst + 1) * 128],
                            xt_t[k][:], start=(k == 0), stop=(k == NE - 1))
                    if dst == 0:
                        nc.vector.tensor_copy(qT_sb[j][0][0:64, :],
                                              d_ps[0:64, :])
                        nc.scalar.copy(qT_sb[j][1][0:64, :], d_ps[64:128, :])
                    elif dst == 1:
                        nc.vector.tensor_copy(kT_sb[j][0][0:64, :],
                                              d_ps[0:64, :])
                        nc.vector.tensor_copy(kT_sb[j][1][0:64, :],
                                              d_ps[64:128, :])
                    else:
                        vt_sb = work.tile([128, 512], bf16, tag='vt', bufs=2)
                        nc.vector.tensor_copy(vt_sb[:], d_ps[:])
                        for sc in range(4):
                            vtr = ps.tile([128, 128], bf16, tag='wei',
                                          bufs=3, name=f'vtr{j}_{sc}')
                            nc.tensor.transpose(
                                vtr[:], vt_sb[:, sc * 128:(sc + 1) * 128],
                                ident[:])
                            nc.vector.tensor_copy(
                                v_sb[j * 4 + sc][:, :, 0:64],
                                vtr.rearrange('p (h e) -> p h e', h=2))

            # ---- B(j): scores + AV accumulation ----
            def emit_B(j):
                b, tb = divmod(j, NTB)
                n_si = 4 * (tb + 1)
                av_pss = [ps.tile([65, 512], f32, tag='av', bufs=3,
                                  name=f'av{j}_{h}') for h in range(HPC)]
                st[j]['av'] = av_pss
                # software-pipelined: AV matmuls trail their scores by LAG
                # groups so the PE never waits on the exp chain
                LAG = 6
                groups = [(si, h) for si in range(n_si) for h in range(HPC)]
                pend = []

                def emit_score(si, h):
                    sblk = b * NTB + si // 4
                    srem = (si % 4) * 128
                    woff = (si - 4 * tb) * 128 if si >= 4 * tb else 0
                    w_ps = ps.tile([128, 512], f32, tag='wei', bufs=3,
                                   name=f'w{j}_{si}_{h}')
                    nc.tensor.matmul(
                        w_ps[:, woff:512],
                        kT_sb[sblk][h][:, srem:srem + 128],
                        qT_sb[j][h][:, woff:512],
                        start=True, stop=True)
                    wt = work.tile([128, 512], bf16, tag='wt', bufs=24,
                                   name=f'wt{j}_{si}_{h}')
                    nc.scalar.activation(wt[:, woff:512], w_ps[:, woff:512],
                                         EXP, scale=SCALE)
                    if si >= 4 * tb:
                        nc.vector.tensor_mul(wt[:, woff:woff + 128],
                                             wt[:, woff:woff + 128],
                                             tmask[:])
                    return (si, h, woff, wt)

                def emit_av(si, h, woff, wt):
                    nc.tensor.matmul(
                        av_pss[h][:, woff:512],
                        v_sb[b * NST + si][:, h, :],
                        wt[:, woff:512],
                        start=(si == 0), stop=(si == n_si - 1),
                        skip_group_check=True)

                for idx, (si, h) in enumerate(groups):
                    pend.append(emit_score(si, h))
                    if idx >= LAG:
                        emit_av(*pend.pop(0))
                for p in pend:
                    emit_av(*p)

            # ---- L(j): stash softmax denominator rows ----
            def emit_L(j):
                lrow = lrow_t[j % 2]
                st[j]['lrow'] = lrow
                with tc.high_priority(offset=120):
                    for h in range(HPC):
                        nc.vector.tensor_copy(lrow[32 * h:32 * h + 1, :],
                                              st[j]['av'][h][64:65, :])

            # ---- M(j): broadcast l, reciprocal, normalize ----
            def emit_M(j):
                bc_ps = ps.tile([128, 512], f32, tag='qkv', bufs=2,
                                name=f'bc{j}')
                with tc.high_priority(offset=120):
                    nc.tensor.matmul(bc_ps[:], sel_bc[:], st[j]['lrow'][:],
                                     start=True, stop=True)
                    rc_sb = work.tile([128, 512], f32, tag='rc', bufs=2)
                    nc.vector.reciprocal(rc_sb[:], bc_ps[:])
                    avT_sb = work.tile([128, 512], bf16, tag='avT', bufs=2)
                    st[j]['avT'] = avT_sb
                    for h in range(HPC):
                        hd = h * 64
                        nc.vector.tensor_mul(avT_sb[hd:hd + 64, :],
                                             st[j]['av'][h][0:64, :],
                                             rc_sb[hd:hd + 64, :])

            # ---- P(j): output projection + DMA ----
            def emit_P(j, final=False):
                t0 = j * 512
                avT_sb = st[j]['avT']
                for tc4 in range(4):
                    for eb in range(2):
                        idx = tc4 * 2 + eb
                        y_ps = ps.tile([128, 512], f32, tag='wei', bufs=3,
                                       name=f'y{j}_{tc4}_{eb}')
                        nc.tensor.matmul(
                            y_ps[:],
                            avT_sb[:, tc4 * 128:(tc4 + 1) * 128],
                            wproj_sb[:, eb * 512:(eb + 1) * 512],
                            start=True, stop=True)
                        y_sb = work.tile([128, 512], f32, tag='ysb', bufs=6)
                        if idx % 4 == 1:
                            nc.scalar.copy(y_sb[:], y_ps[:])
                        else:
                            nc.vector.tensor_copy(y_sb[:], y_ps[:])
                        nc.sync.dma_start(
                            out=y[t0 + tc4 * 128:t0 + (tc4 + 1) * 128,
                                  eb * 512:(eb + 1) * 512],
                            in_=y_sb[:])

            # ---- fused final epilogue: pipeline normalize+proj+copy+DMA
            # per 128-t quarter so the serial tail shrinks
            def emit_MP_final(j):
                t0 = j * 512
                bc_ps = ps.tile([128, 512], f32, tag='qkv', bufs=2,
                                name=f'bcf{j}')
                nc.tensor.matmul(bc_ps[:], sel_bc[:], st[j]['lrow'][:],
                                 start=True, stop=True)
                rc_sb = work.tile([128, 512], f32, tag='rc', bufs=2)
                avT_sb = work.tile([128, 512], bf16, tag='avT', bufs=2)
                for tc4 in range(4):
                    cs = slice(tc4 * 128, (tc4 + 1) * 128)
                    nc.vector.reciprocal(rc_sb[:, cs], bc_ps[:, cs])
                    for h in range(HPC):
                        hd = h * 64
                        nc.vector.tensor_mul(avT_sb[hd:hd + 64, cs],
                                             st[j]['av'][h][0:64, cs],
                                             rc_sb[hd:hd + 64, cs])
                    for eb in range(2):
                        y_ps = ps.tile([128, 512], f32, tag='wei', bufs=3,
                                       name=f'yf{j}_{tc4}_{eb}')
                        nc.tensor.matmul(
                            y_ps[:], avT_sb[:, cs],
                            wproj_sb[:, eb * 512:(eb + 1) * 512],
                            start=True, stop=True)
                        y_sb = work.tile([128, 512], f32, tag='ysb', bufs=6)
                        if eb == 0:
                            nc.vector.tensor_copy(y_sb[:], y_ps[:])
                        else:
                            nc.scalar.copy(y_sb[:], y_ps[:])
                        nc.sync.dma_start(
                            out=y[t0 + tc4 * 128:t0 + (tc4 + 1) * 128,
                                  eb * 512:(eb + 1) * 512],
                            in_=y_sb[:])

            # ---- software-pipelined schedule ----
            # B-order puts the smallest attention block (b1,t0 = j4) last so
            # the un-hideable softmax/proj tail follows a short B phase
            border = [0, 1, 2, 3, 5, 6, 7, 4]
            prev = None
            nextA = 0
            for j in border:
                while nextA <= j:
                    emit_A(nextA)
                    nextA += 1
                if prev is not None:
                    emit_M(prev)
                emit_B(j)
                if prev is not None:
                    emit_P(prev)
                emit_L(j)
                prev = j
            emit_MP_final(prev)

    import concourse.mybir as mybir2
    _split_multi_waits(nc, mybir2)
    return nc


_CACHE = {}


def kernel(x, Wq, Wk, Wv, Wproj, bproj):
    _install_ntff_hook()
    import ml_dtypes
    from concourse.bass_utils import run_bass_kernel_spmd

    bf = ml_dtypes.bfloat16
    x = np.asarray(x, dtype=np.float32)
    Wq = np.asarray(Wq, dtype=np.float32)
    Wk = np.asarray(Wk, dtype=np.float32)
    Wv = np.asarray(Wv, dtype=np.float32)
    Wproj = np.asarray(Wproj, dtype=np.float32)
    bproj = np.asarray(bproj, dtype=np.float32)

    if 'nc' not in _CACHE:
        _CACHE['nc'] = _build_nc()
    nc = _CACHE['nc']

    xT = np.ascontiguousarray(x.reshape(BT, E).T).astype(bf)
    in_maps = []
    for c in range(N_CORES):
        h0 = HPC * c
        wqkv_c = np.concatenate(
            [Wq[h0], Wq[h0 + 1], Wk[h0], Wk[h0 + 1], Wv[h0], Wv[h0 + 1]],
            axis=1)                                         # [E, 384]
        wproj_c = np.ascontiguousarray(Wproj[DPC * c: DPC * (c + 1)])
        in_maps.append({'xt': xT,
                        'wqkv': np.ascontiguousarray(wqkv_c).astype(bf),
                        'wproj': wproj_c.astype(bf)})

    res = run_bass_kernel_spmd(nc, in_maps, list(range(N_CORES)))
    ysum = np.zeros((BT, E), dtype=np.float64)
    for c in range(N_CORES):
        ysum += res.results[c]['y'].astype(np.float64)
    out = (ysum + bproj.astype(np.float64)).astype(np.float32)
    return out.reshape(B, T, E)
